# revision 11
# baseline (speedup 1.0000x reference)
"""Causal multi-head attention (B=4, S=2048, D=1024, H=16, hd=64) on 8
Trainium2 NeuronCores.

Sharding: batch (4-way) x head-group (2-way). Core c handles batch c//2 and
heads [8*(c%2), 8*(c%2)+8). Each core computes its heads' contribution to the
output projection; the host sums the two partials per batch and adds the
bias correction (bv @ Wo + bo; bk is softmax-invariant and dropped; bq is
fused into the Q eviction).

All matmuls in bf16 (1 cycle/row, FWL weight loads). Per-core program:

  Phase A (dense PE, ~82us): stream x^T (bf16, host-cast) and weights into
  SBUF; compute K^T, Q^T (per-hp [2*hd, S] layout) and V (per-key-chunk
  [keys, 8*65] layout with a fused ones column for the softmax denominator)
  for the whole sequence upfront.

  Phase B (per 512-query window, per head-pair): flash-style attention in
  the transposed layout: scoresT = K_chunk @ Q^T as row-tiled concurrent
  pairs (head parities at PE rows 0-63/64-127, outputs in different PSUM
  banks), exp(s/8) split between ACT (exact spline) and DVE (Schraudolph
  int16-bitcast fast exp) so neither engine bottlenecks, causal
  block-skipping plus a triangular mask on diagonal subblocks, attn-out
  accumulated via V_aug matmuls (M=65, Z in row 64), normalized with
  reciprocal_approx_fast + partition_broadcast. Score/V matmuls are
  software-pipelined (zipper).

  Phase C: output projection of window w is deferred into window w+1's
  attention stream so the PE never idles at window boundaries (keeps the
  HAM clock-gate warm).
"""
import numpy as np
import ml_dtypes

import concourse.mybir as mybir
from concourse import bacc
from concourse.tile import TileContext
from concourse.bass_utils import run_bass_kernel_spmd
from concourse.alu_op_type import AluOpType

FP32 = mybir.dt.float32
BF16 = mybir.dt.bfloat16
I16 = mybir.dt.int16
EXPF = mybir.ActivationFunctionType.Exp
IDENT = mybir.ActivationFunctionType.Identity

B, S, D = 4, 2048, 1024
H, HD = 16, 64
NCORES = 8
HPG = 8              # heads per group (per core)
GD = HPG * HD        # 512: group head-dim width
W = 512              # query window
NW = S // W          # 4
KCH = 128            # key chunk
NKC = S // KCH       # 16
DC = 128             # D contraction chunk
NDC = D // DC        # 8
SCALE = 1.0 / 8.0    # 1/sqrt(hd)
GRP = 2              # key chunks per zipper group

# Schraudolph fast exp: bf16_bits(exp(s/8)) ~= round(s * SCH_A + SCH_B)
SCH_A = 128.0 * 1.4426950408889634 / 8.0
SCH_B = 16256.0 - 6.0

_CACHE = {}


def _build_program():
    nc = bacc.Bacc("TRN2", target_bir_lowering=False, debug=False,
                   num_devices=NCORES)

    xT = nc.dram_tensor("xT", [D, S], BF16, kind="ExternalInput").ap()
    wq = nc.dram_tensor("wq", [D, GD], BF16, kind="ExternalInput").ap()
    wk = nc.dram_tensor("wk", [D, GD], BF16, kind="ExternalInput").ap()
    wv = nc.dram_tensor("wv", [D, GD], BF16, kind="ExternalInput").ap()
    wo = nc.dram_tensor("wo", [GD, D], BF16, kind="ExternalInput").ap()
    bq2 = nc.dram_tensor("bq2", [128, 4], FP32, kind="ExternalInput").ap()
    out = nc.dram_tensor("out", [S, D], FP32, kind="ExternalOutput").ap()

    with TileContext(nc) as tc:
        with (
            tc.tile_pool(name="cst", bufs=1) as cst_pool,
            tc.tile_pool(name="wts", bufs=24) as wts_pool,
            tc.tile_pool(name="wo", bufs=4) as wo_pool,
            tc.tile_pool(name="xts", bufs=8) as xts_pool,
            tc.tile_pool(name="kt", bufs=4) as kt_pool,
            tc.tile_pool(name="qt", bufs=4) as qt_pool,
            tc.tile_pool(name="vst", bufs=16) as v_pool,
            tc.tile_pool(name="et", bufs=8) as et_pool,
            tc.tile_pool(name="ao", bufs=8) as ao_pool,
            tc.tile_pool(name="zz", bufs=3) as zz_pool,
            tc.tile_pool(name="ob", bufs=3) as out_pool,
            tc.tile_pool(name="ps", bufs=4, space="PSUM") as ps,
        ):
            # ---- constants ----
            bq_t = cst_pool.tile([128, 4], FP32, tag="bq")
            nc.sync.dma_start(out=bq_t[:], in_=bq2[:])
            trif = cst_pool.tile([128, 128], FP32, tag="trif")
            nc.gpsimd.memset(trif[:], 1.0)
            nc.gpsimd.affine_select(
                out=trif[:], in_=trif[:], compare_op=mybir.AluOpType.is_ge,
                fill=0.0, base=0, pattern=[[1, 128]], channel_multiplier=-1,
            )
            tri = cst_pool.tile([128, 128], BF16, tag="tri")
            nc.vector.tensor_copy(tri[:], trif[:])
            # ACT exp-table warmup (avoid a mid-stream 2.7us table load)
            warm = cst_pool.tile([1, 4], FP32, tag="warm")
            nc.scalar.activation(warm[:], bq_t[0:1, :], EXPF, bias=0.0,
                                 scale=1.0)

            # ---- weight + x DMAs (weights on vector queue, x on sync) ----
            wk_t = [wts_pool.tile([128, GD], BF16, tag="w", name=f"wk{i}")
                    for i in range(NDC)]
            wq_t = [wts_pool.tile([128, GD], BF16, tag="w", name=f"wq{i}")
                    for i in range(NDC)]
            wv_t = [wts_pool.tile([128, GD], BF16, tag="w", name=f"wv{i}")
                    for i in range(NDC)]
            xts = [xts_pool.tile([128, S], BF16, tag="x", name=f"x{i}")
                   for i in range(NDC)]
            for dc in range(NDC):
                nc.gpsimd.dma_start(out=wk_t[dc][:],
                                    in_=wk[dc * DC:(dc + 1) * DC, :])
                nc.sync.dma_start(out=xts[dc][:],
                                  in_=xT[dc * DC:(dc + 1) * DC, :])
            for dc in range(NDC):
                nc.gpsimd.dma_start(out=wq_t[dc][:],
                                    in_=wq[dc * DC:(dc + 1) * DC, :])
            for dc in range(NDC):
                nc.gpsimd.dma_start(out=wv_t[dc][:],
                                    in_=wv[dc * DC:(dc + 1) * DC, :])
            wo_t = [wo_pool.tile([128, D], BF16, tag="wo", name=f"wo{i}")
                    for i in range(4)]
            for hc in range(4):
                nc.gpsimd.dma_start(out=wo_t[hc][:],
                                    in_=wo[hc * 128:(hc + 1) * 128, :])

            # ---- persistent SBUF tensors ----
            kt_tiles = [kt_pool.tile([128, S], BF16, tag="kt", name=f"kt{i}")
                        for i in range(4)]
            qt_tiles = [qt_pool.tile([128, S], BF16, tag="qt", name=f"qt{i}")
                        for i in range(4)]
            v_tiles = [v_pool.tile([128, HPG * 65], BF16, tag="v",
                                   name=f"v{i}") for i in range(NKC)]
            for kc in range(NKC):
                ones_ap = v_tiles[kc][:].rearrange(
                    "p (h e) -> p h e", e=65)[:, :, 64:65]
                nc.gpsimd.memset(ones_ap, 1.0)

            # ---- phase A: K and Q projections (dc-outer, hp-inner) ----
            for dst, wt, is_q in ((kt_tiles, wk_t, False),
                                  (qt_tiles, wq_t, True)):
                for half in range(2):
                    acc = [ps.tile([128, 1024], FP32, tag="ps",
                                   name=f"kq{half}_{i}") for i in range(4)]
                    for dc in range(NDC):
                        for hp in range(4):
                            for nn in range(2):
                                nc.tensor.matmul(
                                    acc[hp][:, nn * 512:nn * 512 + 512],
                                    wt[dc][:, hp * 128:(hp + 1) * 128],
                                    xts[dc][:, half * 1024 + nn * 512:
                                            half * 1024 + nn * 512 + 512],
                                    start=(dc == 0), stop=(dc == NDC - 1))
                    for hp in range(4):
                        dslice = dst[hp][:, half * 1024:half * 1024 + 1024]
                        if is_q:
                            nc.scalar.activation(dslice, acc[hp][:], IDENT,
                                                 bias=bq_t[:, hp:hp + 1],
                                                 scale=1.0)
                        else:
                            nc.scalar.copy(dslice, acc[hp][:])

            # ---- phase A: V projection ----
            for kc in range(NKC):
                v2 = ps.tile([128, 1024], FP32, tag="ps", name=f"v2_{kc}")
                for dc in range(NDC):
                    nc.tensor.matmul(
                        v2[:, 0:512],
                        xts[dc][:, kc * KCH:(kc + 1) * KCH],
                        wv_t[dc][:], start=(dc == 0), stop=(dc == NDC - 1))
                dstv = v_tiles[kc][:].rearrange(
                    "p (h e) -> p h e", e=65)[:, :, 0:64]
                srcv = v2[:, 0:512].rearrange("p (h e) -> p h e", e=64)
                nc.vector.tensor_copy(dstv, srcv)

            # ---- phases B/C ----
            exp_ctr = [0]

            def emit_S(w, hp, kcs):
                ets = {}
                for kc in kcs:
                    j = kc - 4 * w
                    lo = max(j, 0) * 128
                    s2 = ps.tile([128, 1024], FP32, tag="ps", name="s2")
                    et = et_pool.tile([128, 1024], BF16, tag="et")
                    for par in range(2):
                        nc.tensor.matmul(
                            s2[:, par * 512 + lo:par * 512 + 512],
                            kt_tiles[hp][par * 64:(par + 1) * 64,
                                         kc * KCH:(kc + 1) * KCH],
                            qt_tiles[hp][par * 64:(par + 1) * 64,
                                         w * W + lo:w * W + W],
                            start=True, stop=True)
                    if lo == 0:
                        src, dst = s2[:], et[:]
                    else:
                        src = s2[:].rearrange("p (two n) -> p two n",
                                              two=2)[:, :, lo:512]
                        dst = et[:].rearrange("p (two n) -> p two n",
                                              two=2)[:, :, lo:512]
                    sel = 0 if j >= 0 else exp_ctr[0] % 3
                    if j < 0:
                        exp_ctr[0] += 1
                    if sel == 0:
                        nc.scalar.activation(dst, src, EXPF, bias=0.0,
                                             scale=SCALE)
                    elif sel == 1:
                        nc.vector.tensor_scalar(
                            dst.bitcast(I16), src, SCH_A, SCH_B,
                            AluOpType.mult, AluOpType.add)
                    else:
                        nc.vector.tensor_scalar(
                            dst.bitcast(I16), src, SCH_A, SCH_B,
                            AluOpType.mult, AluOpType.add)
                    if j >= 0:
                        for par in range(2):
                            seg = et[:, par * 512 + lo:par * 512 + lo + 128]
                            nc.vector.tensor_mul(seg, seg, tri[:])
                    ets[kc] = et
                return ets

            def emit_V(w, hp, o2, kcs, ets, nkc):
                for kc in kcs:
                    j = kc - 4 * w
                    lo = max(j, 0) * 128
                    for par in range(2):
                        h = 2 * hp + par
                        nc.tensor.matmul(
                            o2[0:65, par * 512 + lo:par * 512 + 512],
                            v_tiles[kc][:, h * 65:(h + 1) * 65],
                            ets[kc][:, par * 512 + lo:par * 512 + 512],
                            start=(kc == 0), stop=(kc == nkc - 1))

            def emit_norm(w, hp, o2, ao_w):
                ao = ao_pool.tile([128, W], BF16, tag="ao",
                                  name=f"ao{w}_{hp}")
                # reciprocal_approx_fast requires SBUF input at base
                # partition 0; stage both parities' Z rows in one shot
                zc = zz_pool.tile([1, 2 * W], FP32, tag="zc")
                nc.vector.tensor_copy(zc[:], o2[64:65, :])
                zr = zz_pool.tile([1, 2 * W], FP32, tag="zr")
                nc.vector.reciprocal_approx_fast(zr[:], zc[:])
                zb = zz_pool.tile([64, 2 * W], FP32, tag="zb")
                nc.gpsimd.partition_broadcast(zb[:], zr[:])
                for par in range(2):
                    nc.vector.tensor_mul(
                        ao[par * 64:(par + 1) * 64, :],
                        o2[0:65, par * 512:par * 512 + 512][0:64, :],
                        zb[:, par * 512:par * 512 + 512])
                ao_w[hp] = ao

            def emit_op_chunk(w, qs, ao_w):
                op2 = ps.tile([128, 1024], FP32, tag="ps", name="op2")
                for dcol in range(2):
                    for hc in range(4):
                        nc.tensor.matmul(
                            op2[:, dcol * 512:dcol * 512 + 512],
                            ao_w[hc][:, qs * 128:(qs + 1) * 128],
                            wo_t[hc][:, dcol * 512:dcol * 512 + 512],
                            start=(hc == 0), stop=(hc == 3))
                ot = out_pool.tile([128, 1024], FP32, tag="ob")
                nc.vector.tensor_copy(ot[:], op2[:])
                nc.sync.dma_start(
                    out=out[w * W + qs * 128:w * W + (qs + 1) * 128, :],
                    in_=ot[:])

            op_q = []           # deferred out-proj chunks: (w, qs, ao_w)
            for w in range(NW):
                nkc = 4 * (w + 1)
                ao_w = {}
                o2s = {}
                pending = None
                gcount = 0
                for hp in range(4):
                    o2s[hp] = ps.tile([128, 1024], FP32, tag="ps",
                                      name=f"o2_{w}_{hp}")
                    for kc0 in range(0, nkc, GRP):
                        kcs = list(range(kc0, min(kc0 + GRP, nkc)))
                        ets = emit_S(w, hp, kcs)
                        if pending is not None:
                            p_hp, p_kcs, p_ets = pending
                            emit_V(w, p_hp, o2s[p_hp], p_kcs, p_ets, nkc)
                        pending = (hp, kcs, ets)
                        # norm for the previous hp, one group late so its
                        # o2-stop matmul has drained (avoids queue convoys)
                        if kc0 == GRP and hp > 0:
                            emit_norm(w, hp - 1, o2s[hp - 1], ao_w)
                        gcount += 1
                        if op_q and gcount >= 3:
                            emit_op_chunk(*op_q.pop(0))
                p_hp, p_kcs, p_ets = pending
                emit_V(w, p_hp, o2s[p_hp], p_kcs, p_ets, nkc)
                emit_norm(w, p_hp, o2s[p_hp], ao_w)
                for item in op_q:   # leftovers (shouldn't happen for w>0)
                    emit_op_chunk(*item)
                op_q = [(w, qs, ao_w) for qs in range(4)]
            for item in op_q:
                emit_op_chunk(*item)

    nc.compile()
    return nc


def _get_program():
    if "nc" not in _CACHE:
        _CACHE["nc"] = _build_program()
    return _CACHE["nc"]


def _install_ntff_hook():
    """The agent image's antenv lacks axon_hooks; shim it and register the
    ctypes NTFF profiling hook so trace=True yields exec_time_ns."""
    import sys, types
    if "antenv.axon_hooks" in sys.modules:
        return
    try:
        import antenv
        mod = types.ModuleType("antenv.axon_hooks")
        _h = [None]
        mod.set_axon_ntff_profile_hook = lambda h: _h.__setitem__(0, h)
        mod.get_axon_ntff_profile_hook = lambda: _h[0]
        sys.modules["antenv.axon_hooks"] = mod
        antenv.axon_hooks = mod
        from trn_agent_boot.trn_boot import _ntff_profile_via_ctypes
        mod.set_axon_ntff_profile_hook(
            _ntff_profile_via_ctypes("/opt/axon/libaxon_pjrt.so"))
    except Exception as e:  # degrade: run without tracing
        print(f"NTFF hook install failed ({e}); tracing disabled")


def _run(inputs, trace=False):
    bf16 = ml_dtypes.bfloat16
    x = np.asarray(inputs["x"], dtype=np.float32)
    Wq = np.asarray(inputs["Wq"], dtype=np.float32)
    Wk = np.asarray(inputs["Wk"], dtype=np.float32)
    Wv = np.asarray(inputs["Wv"], dtype=np.float32)
    Wo = np.asarray(inputs["Wo"], dtype=np.float32)
    bq = np.asarray(inputs["bq"], dtype=np.float32)
    bv = np.asarray(inputs["bv"], dtype=np.float32)
    bo = np.asarray(inputs["bo"], dtype=np.float32)

    if trace:
        _install_ntff_hook()
    nc = _get_program()
    xb = [np.ascontiguousarray(x[b].T).astype(bf16) for b in range(B)]
    in_maps = []
    for c in range(NCORES):
        b, g = divmod(c, 2)
        sl = slice(g * GD, (g + 1) * GD)
        in_maps.append({
            "xT": xb[b],
            "wq": np.ascontiguousarray(Wq[:, sl]).astype(bf16),
            "wk": np.ascontiguousarray(Wk[:, sl]).astype(bf16),
            "wv": np.ascontiguousarray(Wv[:, sl]).astype(bf16),
            "wo": np.ascontiguousarray(Wo[sl, :]).astype(bf16),
            "bq2": np.ascontiguousarray(bq[sl].reshape(4, 128).T),
        })
    res = run_bass_kernel_spmd(nc, in_maps, list(range(NCORES)), trace=trace)
    outp = np.empty((B, S, D), dtype=np.float32)
    # bias correction: bk shifts all logits of a query equally (softmax-
    # invariant, dropped); attention rows sum to 1 so bv flows through Wo
    # as a constant row vector; bq is fused on-device.
    corr = (bv @ Wo + bo).astype(np.float32)
    for b in range(B):
        outp[b] = res.results[2 * b]["out"] + res.results[2 * b + 1]["out"] + corr
    return outp, res


def kernel(**inputs):
    outp, _ = _run(inputs, trace=False)
    return outp


def kernel_traced(**inputs):
    outp, res = _run(inputs, trace=True)
    return outp, res


# revision 12
# speedup vs baseline: 1.0821x; 1.0821x over previous
"""Causal multi-head attention (B=4, S=2048, D=1024, H=16, hd=64) on 8
Trainium2 NeuronCores.

Sharding: batch (4-way) x head-group (2-way). Core c handles batch c//2 and
heads [8*(c%2), 8*(c%2)+8). Each core computes its heads' contribution to the
output projection; the host sums the two partials per batch and adds the
bias correction (bv @ Wo + bo; bk is softmax-invariant and dropped; bq is
fused into the Q eviction).

All matmuls in bf16 (1 cycle/row, FWL weight loads). Per-core program:

  Phase A (dense PE, ~82us): stream x^T (bf16, host-cast) and weights into
  SBUF; compute K^T, Q^T (per-hp [2*hd, S] layout) and V (per-key-chunk
  [keys, 8*65] layout with a fused ones column for the softmax denominator)
  for the whole sequence upfront.

  Phase B (per 512-query window, per head-pair): flash-style attention in
  the transposed layout: scoresT = K_chunk @ Q^T as row-tiled concurrent
  pairs (head parities at PE rows 0-63/64-127, outputs in different PSUM
  banks), exp(s/8) split between ACT (exact spline) and DVE (Schraudolph
  int16-bitcast fast exp) so neither engine bottlenecks, causal
  block-skipping plus a triangular mask on diagonal subblocks, attn-out
  accumulated via V_aug matmuls (M=65, Z in row 64), normalized with
  reciprocal_approx_fast + partition_broadcast. Score/V matmuls are
  software-pipelined (zipper).

  Phase C: output projection of window w is deferred into window w+1's
  attention stream so the PE never idles at window boundaries (keeps the
  HAM clock-gate warm).
"""
import numpy as np
import ml_dtypes

import concourse.mybir as mybir
from concourse import bacc
from concourse.tile import TileContext
from concourse.bass_utils import run_bass_kernel_spmd
from concourse.alu_op_type import AluOpType

FP32 = mybir.dt.float32
BF16 = mybir.dt.bfloat16
I16 = mybir.dt.int16
EXPF = mybir.ActivationFunctionType.Exp
IDENT = mybir.ActivationFunctionType.Identity

B, S, D = 4, 2048, 1024
H, HD = 16, 64
NCORES = 8
HPG = 8              # heads per group (per core)
GD = HPG * HD        # 512: group head-dim width
W = 512              # query window
NW = S // W          # 4
KCH = 128            # key chunk
NKC = S // KCH       # 16
DC = 128             # D contraction chunk
NDC = D // DC        # 8
SCALE = 1.0 / 8.0    # 1/sqrt(hd)
GRP = 2              # key chunks per zipper group

# Schraudolph fast exp: bf16_bits(exp(s/8)) ~= round(s * SCH_A + SCH_B)
SCH_A = 128.0 * 1.4426950408889634 / 8.0
SCH_B = 16256.0 - 6.0

_CACHE = {}


def _build_program():
    nc = bacc.Bacc("TRN2", target_bir_lowering=False, debug=False,
                   num_devices=NCORES)

    xT = nc.dram_tensor("xT", [D, S], BF16, kind="ExternalInput").ap()
    wq = nc.dram_tensor("wq", [D, GD], BF16, kind="ExternalInput").ap()
    wk = nc.dram_tensor("wk", [D, GD], BF16, kind="ExternalInput").ap()
    wv = nc.dram_tensor("wv", [D, GD], BF16, kind="ExternalInput").ap()
    wo = nc.dram_tensor("wo", [GD, D], BF16, kind="ExternalInput").ap()
    bq2 = nc.dram_tensor("bq2", [128, 4], FP32, kind="ExternalInput").ap()
    out = nc.dram_tensor("out", [S, D], FP32, kind="ExternalOutput").ap()

    with TileContext(nc) as tc:
        with (
            tc.tile_pool(name="cst", bufs=1) as cst_pool,
            tc.tile_pool(name="wts", bufs=24) as wts_pool,
            tc.tile_pool(name="wo", bufs=4) as wo_pool,
            tc.tile_pool(name="xts", bufs=8) as xts_pool,
            tc.tile_pool(name="kt", bufs=4) as kt_pool,
            tc.tile_pool(name="qt", bufs=4) as qt_pool,
            tc.tile_pool(name="vst", bufs=16) as v_pool,
            tc.tile_pool(name="et", bufs=8) as et_pool,
            tc.tile_pool(name="ao", bufs=8) as ao_pool,
            tc.tile_pool(name="zz", bufs=3) as zz_pool,
            tc.tile_pool(name="ob", bufs=3) as out_pool,
            tc.tile_pool(name="ps", bufs=4, space="PSUM") as ps,
        ):
            # ---- constants ----
            bq_t = cst_pool.tile([128, 4], FP32, tag="bq")
            nc.sync.dma_start(out=bq_t[:], in_=bq2[:])
            trif = cst_pool.tile([128, 128], FP32, tag="trif")
            nc.gpsimd.memset(trif[:], 1.0)
            nc.gpsimd.affine_select(
                out=trif[:], in_=trif[:], compare_op=mybir.AluOpType.is_ge,
                fill=0.0, base=0, pattern=[[1, 128]], channel_multiplier=-1,
            )
            tri = cst_pool.tile([128, 128], BF16, tag="tri")
            nc.vector.tensor_copy(tri[:], trif[:])
            # ACT exp-table warmup (avoid a mid-stream 2.7us table load)
            warm = cst_pool.tile([1, 4], FP32, tag="warm")
            nc.scalar.activation(warm[:], bq_t[0:1, :], EXPF, bias=0.0,
                                 scale=1.0)

            # ---- weight + x DMAs (weights on vector queue, x on sync) ----
            wk_t = [wts_pool.tile([128, GD], BF16, tag="w", name=f"wk{i}")
                    for i in range(NDC)]
            wq_t = [wts_pool.tile([128, GD], BF16, tag="w", name=f"wq{i}")
                    for i in range(NDC)]
            wv_t = [wts_pool.tile([128, GD], BF16, tag="w", name=f"wv{i}")
                    for i in range(NDC)]
            xts = [xts_pool.tile([128, S], BF16, tag="x", name=f"x{i}")
                   for i in range(NDC)]
            for dc in range(NDC):
                nc.gpsimd.dma_start(out=wk_t[dc][:],
                                    in_=wk[dc * DC:(dc + 1) * DC, :])
                nc.sync.dma_start(out=xts[dc][:],
                                  in_=xT[dc * DC:(dc + 1) * DC, :])
            for dc in range(NDC):
                nc.gpsimd.dma_start(out=wq_t[dc][:],
                                    in_=wq[dc * DC:(dc + 1) * DC, :])
            for dc in range(NDC):
                nc.gpsimd.dma_start(out=wv_t[dc][:],
                                    in_=wv[dc * DC:(dc + 1) * DC, :])
            wo_t = [wo_pool.tile([128, D], BF16, tag="wo", name=f"wo{i}")
                    for i in range(4)]
            for hc in range(4):
                nc.gpsimd.dma_start(out=wo_t[hc][:],
                                    in_=wo[hc * 128:(hc + 1) * 128, :])

            # ---- persistent SBUF tensors ----
            kt_tiles = [kt_pool.tile([128, S], BF16, tag="kt", name=f"kt{i}")
                        for i in range(4)]
            qt_tiles = [qt_pool.tile([128, S], BF16, tag="qt", name=f"qt{i}")
                        for i in range(4)]
            v_tiles = [v_pool.tile([128, HPG * 65], BF16, tag="v",
                                   name=f"v{i}") for i in range(NKC)]
            for kc in range(NKC):
                ones_ap = v_tiles[kc][:].rearrange(
                    "p (h e) -> p h e", e=65)[:, :, 64:65]
                nc.gpsimd.memset(ones_ap, 1.0)

            # ---- phase A: K and Q projections (dc-outer, hp-inner) ----
            for dst, wt, is_q in ((kt_tiles, wk_t, False),
                                  (qt_tiles, wq_t, True)):
                for half in range(2):
                    acc = [ps.tile([128, 1024], FP32, tag="ps",
                                   name=f"kq{half}_{i}") for i in range(4)]
                    for dc in range(NDC):
                        for hp in range(4):
                            for nn in range(2):
                                nc.tensor.matmul(
                                    acc[hp][:, nn * 512:nn * 512 + 512],
                                    wt[dc][:, hp * 128:(hp + 1) * 128],
                                    xts[dc][:, half * 1024 + nn * 512:
                                            half * 1024 + nn * 512 + 512],
                                    start=(dc == 0), stop=(dc == NDC - 1))
                    for hp in range(4):
                        dslice = dst[hp][:, half * 1024:half * 1024 + 1024]
                        if is_q:
                            nc.scalar.activation(dslice, acc[hp][:], IDENT,
                                                 bias=bq_t[:, hp:hp + 1],
                                                 scale=1.0)
                        else:
                            nc.scalar.copy(dslice, acc[hp][:])

            # ---- phase A: V projection ----
            for kc in range(NKC):
                v2 = ps.tile([128, 1024], FP32, tag="ps", name=f"v2_{kc}")
                for dc in range(NDC):
                    nc.tensor.matmul(
                        v2[:, 0:512],
                        xts[dc][:, kc * KCH:(kc + 1) * KCH],
                        wv_t[dc][:], start=(dc == 0), stop=(dc == NDC - 1))
                dstv = v_tiles[kc][:].rearrange(
                    "p (h e) -> p h e", e=65)[:, :, 0:64]
                srcv = v2[:, 0:512].rearrange("p (h e) -> p h e", e=64)
                nc.vector.tensor_copy(dstv, srcv)

            # ---- phases B/C ----
            exp_ctr = [0]

            def emit_S(w, hp, kcs):
                ets = {}
                for kc in kcs:
                    j = kc - 4 * w
                    lo = max(j, 0) * 128
                    s2 = ps.tile([128, 1024], FP32, tag="ps", name="s2")
                    et = et_pool.tile([128, 1024], BF16, tag="et")
                    for par in range(2):
                        nc.tensor.matmul(
                            s2[:, par * 512 + lo:par * 512 + 512],
                            kt_tiles[hp][par * 64:(par + 1) * 64,
                                         kc * KCH:(kc + 1) * KCH],
                            qt_tiles[hp][par * 64:(par + 1) * 64,
                                         w * W + lo:w * W + W],
                            start=True, stop=True)
                    if lo == 0:
                        src, dst = s2[:], et[:]
                    else:
                        src = s2[:].rearrange("p (two n) -> p two n",
                                              two=2)[:, :, lo:512]
                        dst = et[:].rearrange("p (two n) -> p two n",
                                              two=2)[:, :, lo:512]
                    use_act = j >= 0 or exp_ctr[0] % 3 != 2
                    if j < 0:
                        exp_ctr[0] += 1
                    if use_act:
                        nc.scalar.activation(dst, src, EXPF, bias=0.0,
                                             scale=SCALE)
                    else:
                        nc.vector.tensor_scalar(
                            dst.bitcast(I16), src, SCH_A, SCH_B,
                            AluOpType.mult, AluOpType.add)
                    if j >= 0:
                        for par in range(2):
                            seg = et[:, par * 512 + lo:par * 512 + lo + 128]
                            nc.vector.tensor_mul(seg, seg, tri[:])
                    ets[kc] = et
                return ets

            def emit_V(w, hp, o2, kcs, ets, nkc):
                for kc in kcs:
                    j = kc - 4 * w
                    lo = max(j, 0) * 128
                    for par in range(2):
                        h = 2 * hp + par
                        nc.tensor.matmul(
                            o2[0:65, par * 512 + lo:par * 512 + 512],
                            v_tiles[kc][:, h * 65:(h + 1) * 65],
                            ets[kc][:, par * 512 + lo:par * 512 + 512],
                            start=(kc == 0), stop=(kc == nkc - 1))

            def emit_norm(w, hp, o2, ao_w):
                ao = ao_pool.tile([128, W], BF16, tag="ao",
                                  name=f"ao{w}_{hp}")
                # reciprocal_approx_fast requires SBUF input at base
                # partition 0; stage both parities' Z rows in one shot
                zc = zz_pool.tile([1, 2 * W], FP32, tag="zc")
                nc.vector.tensor_copy(zc[:], o2[64:65, :])
                zr = zz_pool.tile([1, 2 * W], FP32, tag="zr")
                nc.vector.reciprocal_approx_fast(zr[:], zc[:])
                zb = zz_pool.tile([64, 2 * W], FP32, tag="zb")
                nc.gpsimd.partition_broadcast(zb[:], zr[:])
                for par in range(2):
                    nc.vector.tensor_mul(
                        ao[par * 64:(par + 1) * 64, :],
                        o2[0:65, par * 512:par * 512 + 512][0:64, :],
                        zb[:, par * 512:par * 512 + 512])
                ao_w[hp] = ao

            def emit_op_chunk(w, qs, ao_w):
                op2 = ps.tile([128, 1024], FP32, tag="ps", name="op2")
                for dcol in range(2):
                    for hc in range(4):
                        nc.tensor.matmul(
                            op2[:, dcol * 512:dcol * 512 + 512],
                            ao_w[hc][:, qs * 128:(qs + 1) * 128],
                            wo_t[hc][:, dcol * 512:dcol * 512 + 512],
                            start=(hc == 0), stop=(hc == 3))
                ot = out_pool.tile([128, 1024], FP32, tag="ob")
                nc.vector.tensor_copy(ot[:], op2[:])
                nc.sync.dma_start(
                    out=out[w * W + qs * 128:w * W + (qs + 1) * 128, :],
                    in_=ot[:])

            op_q = []           # deferred out-proj chunks: (w, qs, ao_w)
            for w in range(NW):
                nkc = 4 * (w + 1)
                ao_w = {}
                o2s = {}
                pending = None
                gcount = 0
                for hp in range(4):
                    o2s[hp] = ps.tile([128, 1024], FP32, tag="ps",
                                      name=f"o2_{w}_{hp}")
                    for kc0 in range(0, nkc, GRP):
                        kcs = list(range(kc0, min(kc0 + GRP, nkc)))
                        ets = emit_S(w, hp, kcs)
                        if pending is not None:
                            p_hp, p_kcs, p_ets = pending
                            emit_V(w, p_hp, o2s[p_hp], p_kcs, p_ets, nkc)
                        pending = (hp, kcs, ets)
                        # norm for the previous hp, one group late so its
                        # o2-stop matmul has drained (avoids queue convoys)
                        if kc0 == GRP and hp > 0:
                            emit_norm(w, hp - 1, o2s[hp - 1], ao_w)
                        gcount += 1
                        if op_q and gcount >= 3:
                            emit_op_chunk(*op_q.pop(0))
                p_hp, p_kcs, p_ets = pending
                emit_V(w, p_hp, o2s[p_hp], p_kcs, p_ets, nkc)
                emit_norm(w, p_hp, o2s[p_hp], ao_w)
                for item in op_q:   # leftovers (shouldn't happen for w>0)
                    emit_op_chunk(*item)
                op_q = [(w, qs, ao_w) for qs in range(4)]
            for item in op_q:
                emit_op_chunk(*item)

    nc.compile()
    return nc


def _get_program():
    if "nc" not in _CACHE:
        _CACHE["nc"] = _build_program()
    return _CACHE["nc"]


def _install_ntff_hook():
    """The agent image's antenv lacks axon_hooks; shim it and register the
    ctypes NTFF profiling hook so trace=True yields exec_time_ns."""
    import sys, types
    if "antenv.axon_hooks" in sys.modules:
        return
    try:
        import antenv
        mod = types.ModuleType("antenv.axon_hooks")
        _h = [None]
        mod.set_axon_ntff_profile_hook = lambda h: _h.__setitem__(0, h)
        mod.get_axon_ntff_profile_hook = lambda: _h[0]
        sys.modules["antenv.axon_hooks"] = mod
        antenv.axon_hooks = mod
        from trn_agent_boot.trn_boot import _ntff_profile_via_ctypes
        mod.set_axon_ntff_profile_hook(
            _ntff_profile_via_ctypes("/opt/axon/libaxon_pjrt.so"))
    except Exception as e:  # degrade: run without tracing
        print(f"NTFF hook install failed ({e}); tracing disabled")


def _run(inputs, trace=False):
    bf16 = ml_dtypes.bfloat16
    x = np.asarray(inputs["x"], dtype=np.float32)
    Wq = np.asarray(inputs["Wq"], dtype=np.float32)
    Wk = np.asarray(inputs["Wk"], dtype=np.float32)
    Wv = np.asarray(inputs["Wv"], dtype=np.float32)
    Wo = np.asarray(inputs["Wo"], dtype=np.float32)
    bq = np.asarray(inputs["bq"], dtype=np.float32)
    bv = np.asarray(inputs["bv"], dtype=np.float32)
    bo = np.asarray(inputs["bo"], dtype=np.float32)

    if trace:
        _install_ntff_hook()
    nc = _get_program()
    xb = [np.ascontiguousarray(x[b].T).astype(bf16) for b in range(B)]
    in_maps = []
    for c in range(NCORES):
        b, g = divmod(c, 2)
        sl = slice(g * GD, (g + 1) * GD)
        in_maps.append({
            "xT": xb[b],
            "wq": np.ascontiguousarray(Wq[:, sl]).astype(bf16),
            "wk": np.ascontiguousarray(Wk[:, sl]).astype(bf16),
            "wv": np.ascontiguousarray(Wv[:, sl]).astype(bf16),
            "wo": np.ascontiguousarray(Wo[sl, :]).astype(bf16),
            "bq2": np.ascontiguousarray(bq[sl].reshape(4, 128).T),
        })
    res = run_bass_kernel_spmd(nc, in_maps, list(range(NCORES)), trace=trace)
    outp = np.empty((B, S, D), dtype=np.float32)
    # bias correction: bk shifts all logits of a query equally (softmax-
    # invariant, dropped); attention rows sum to 1 so bv flows through Wo
    # as a constant row vector; bq is fused on-device.
    corr = (bv @ Wo + bo).astype(np.float32)
    for b in range(B):
        outp[b] = res.results[2 * b]["out"] + res.results[2 * b + 1]["out"] + corr
    return outp, res


def kernel(**inputs):
    outp, _ = _run(inputs, trace=False)
    return outp


def kernel_traced(**inputs):
    outp, res = _run(inputs, trace=True)
    return outp, res
